# revision 7
# baseline (speedup 1.0000x reference)
"""AccumulateNeighbours (meanmax) Trainium2 kernel.

out[v] = concat(mean_k feat[nidx[v,k]], max_k feat[nidx[v,k]])  -> [V, 2F]

Design (see probes in git history of this session):
- Vertices sharded over 8 NeuronCores; fp16 feature table replicated per core.
- Gather uses the vectorized SWDGE `dma_gather` (InstDMAGatherAnt): int16
  indices => the 150k-row table is split into 5 windows of 30000 rows, each
  prefixed by an all-zero row (the padding target, local idx 0). Rows are
  padded to 256B (128 fp16). Values are stored as fp16(x + 8) so zero pads
  are neutral: sum gets +0 (corrected by the constant 32*8), max is dominated
  (x+8 > 0).
- Position coding: for a 128-vertex tile and window w, the index list is
  [C_w, 128] (list position c*128+p = neighbour c of vertex-in-partition p),
  padded per vertex to the tile max C_w. To shrink the padding, vertices are
  globally re-ordered (clustered by window-count profile) and dealt into
  tiles of 1024 (128 to each core) sharing one schedule; the host undoes the
  permutation on output.
- dma_gather is limited to 1024 indices (idx-read shape field): each window
  gather is chunked into <=8-column pieces. Chunks rotate over the 4 SWDGE
  queues; descriptor generation pipelines 4-wide across Q7 core pairs
  (~8ns/idx per pair, ~2ns/idx effective).
- Reductions: pairwise trees over the gathered [128, CT, 128] fp16 tile on
  the Vector engine (contiguous views), final level writes fp32 into the
  [128, 192] out tile. Host: mean = sum/32 - 8, max -= 8.

The schedule depends on the index data, so the device program is built per
input (compile is host-side, cached on the schedule bytes).
"""

import numpy as np

import concourse.bacc as bacc
import concourse.mybir as mybir
import concourse.tile as tile
from concourse import bass_utils

V, K, F = 150000, 32, 96
NCORES = 8
VS = V // NCORES  # 18750
P = 128
NT = (VS + P - 1) // P  # 147 (last tile 62 rows/core, 496-vertex supertile)
NW = 5
WROWS = 30000
TROWS = NW * (WROWS + 1)
RE = 128  # 256B rows
BIAS = 8.0
CMAX = 8  # max columns per dma_gather (1024 idx limit)

G_BUFS = 4
IDX_BUFS = 6
OUT_BUFS = 6
NQUEUES = 4

_prog_cache: dict = {}


def _build(sched: np.ndarray, idx_total: int):
    nc = bacc.Bacc(
        "TRN2", target_bir_lowering=False, debug=False, num_swdge_queues=NQUEUES
    )
    tab_d = nc.dram_tensor("tab", [TROWS, RE], mybir.dt.float16, kind="ExternalInput")
    idx_d = nc.dram_tensor("idxb", [P, idx_total], mybir.dt.int16, kind="ExternalInput")
    out_d = nc.dram_tensor("out", [VS, 2 * F], mybir.dt.float32, kind="ExternalOutput")

    tab_ap = tab_d.ap()
    idx_ap = idx_d.ap()
    out_ap = out_d.ap()

    cts = sched.sum(axis=1)
    ctm = int(cts.max())
    sa_w = ctm // 2 + 1
    sb_w = ctm // 4 + 2

    qctr = 0
    with tile.TileContext(nc) as tc:
        with (
            tc.tile_pool(name="idx", bufs=IDX_BUFS) as idx_pool,
            tc.tile_pool(name="g", bufs=G_BUFS) as g_pool,
            tc.tile_pool(name="sa", bufs=2) as sa_pool,
            tc.tile_pool(name="sb", bufs=2) as sb_pool,
            tc.tile_pool(name="ma", bufs=2) as ma_pool,
            tc.tile_pool(name="mb", bufs=2) as mb_pool,
            tc.tile_pool(name="o", bufs=OUT_BUFS) as o_pool,
        ):
            ioff = 0
            for t in range(NT):
                rows = min(P, VS - t * P)
                ct = int(cts[t])
                iw = 8 * ct
                idx_tile = idx_pool.tile([P, iw], mybir.dt.int16)
                nc.sync.dma_start(idx_tile[:, :], idx_ap[:, ioff : ioff + iw])

                g_tile = g_pool.tile([P, ct, RE], mybir.dt.float16)
                off = 0
                for w in range(NW):
                    cw = int(sched[t, w])
                    done = 0
                    while done < cw:
                        cc = min(CMAX, cw - done)
                        n = cc * P
                        c0 = off + done
                        nc.gpsimd.dma_gather(
                            out_ap=g_tile[:, c0 : c0 + cc, :],
                            in_ap=tab_ap[w * (WROWS + 1) : (w + 1) * (WROWS + 1), :],
                            idxs_ap=idx_tile[:, c0 * 8 : (c0 + cc) * 8],
                            num_idxs=n,
                            num_idxs_reg=n,
                            elem_size=RE,
                            elem_step=RE,
                            queue_num=qctr % NQUEUES,
                            single_packet=False,
                        )
                        qctr += 1
                        done += cc
                    off += cw

                o_tile = o_pool.tile([P, 2 * F], mybir.dt.float32)

                # sum tree (exact pairing; odd level copies the straggler)
                sa = sa_pool.tile([P, sa_w, RE], mybir.dt.float16)
                sb = sb_pool.tile([P, sb_w, RE], mybir.dt.float16)
                cur, nxt = None, sa
                n = ct
                while n > 2:
                    src = g_tile if cur is None else cur
                    h = n // 2
                    nc.vector.tensor_add(
                        nxt[:, 0:h, :], src[:, 0:h, :], src[:, h : 2 * h, :]
                    )
                    if n % 2:
                        nc.scalar.copy(
                            nxt[:, h : h + 1, :], src[:, 2 * h : 2 * h + 1, :]
                        )
                    n = h + (n % 2)
                    cur, nxt = nxt, (sb if nxt is sa else sa)
                src = g_tile if cur is None else cur
                if n == 2:
                    nc.vector.tensor_add(
                        o_tile[:, 0:F], src[:, 0, 0:F], src[:, 1, 0:F]
                    )
                else:
                    nc.scalar.copy(o_tile[:, 0:F], src[:, 0, 0:F])

                # max tree (overlap pairing handles odd)
                ma = ma_pool.tile([P, sa_w, RE], mybir.dt.float16)
                mb = mb_pool.tile([P, sb_w, RE], mybir.dt.float16)
                cur, nxt = None, ma
                n = ct
                while n > 2:
                    src = g_tile if cur is None else cur
                    h = (n + 1) // 2
                    nc.vector.tensor_max(
                        nxt[:, 0:h, :], src[:, 0:h, :], src[:, n - h : n, :]
                    )
                    n = h
                    cur, nxt = nxt, (mb if nxt is ma else ma)
                src = g_tile if cur is None else cur
                if n == 2:
                    nc.vector.tensor_max(
                        o_tile[:, F : 2 * F], src[:, 0, 0:F], src[:, 1, 0:F]
                    )
                else:
                    nc.scalar.copy(o_tile[:, F : 2 * F], src[:, 0, 0:F])

                nc.sync.dma_start(out_ap[t * P : t * P + rows, :], o_tile[:rows, :])
                ioff += iw
    nc.compile()
    return nc


def _get_prog(sched: np.ndarray, idx_total: int):
    key = (sched.tobytes(), idx_total, G_BUFS, IDX_BUFS, NQUEUES, CMAX, "v5buf")
    if key not in _prog_cache:
        _prog_cache[key] = _build(sched, idx_total)
    return _prog_cache[key]


def _prepare(feat: np.ndarray, nidx: np.ndarray):
    tab = np.zeros((TROWS, RE), dtype=np.float16)
    f16 = (feat + BIAS).astype(np.float16)
    for w in range(NW):
        tab[w * (WROWS + 1) + 1 : (w + 1) * (WROWS + 1), :F] = f16[
            w * WROWS : (w + 1) * WROWS
        ]

    win32 = (nidx // WROWS).astype(np.int32)  # [V, K]
    loc = (nidx - win32.astype(np.int64) * WROWS + 1).astype(np.int16)

    cnt = np.stack([(win32 == w).sum(axis=1) for w in range(NW)], axis=1).astype(
        np.int32
    )  # [V, NW]

    # cluster vertices by window profile; deal into supertiles of 1024
    snd = np.sort(cnt, axis=1)[:, -2]
    key = cnt.argmax(axis=1) * 10000 + cnt.max(axis=1) * 100 + snd
    order = np.argsort(key, kind="stable")  # dev position -> original vertex

    # dev2orig[c, r] = original vertex of core c, device row r
    dev2orig = np.empty((NCORES, VS), dtype=np.int64)
    nfull = VS // P  # 146 full tiles
    tail = VS - nfull * P  # 62
    for t in range(nfull):
        blk = order[t * NCORES * P : (t + 1) * NCORES * P]
        for c in range(NCORES):
            dev2orig[c, t * P : (t + 1) * P] = blk[c * P : (c + 1) * P]
    blk = order[nfull * NCORES * P :]
    for c in range(NCORES):
        dev2orig[c, nfull * P :] = blk[c * tail : (c + 1) * tail]

    # schedule: per supertile, max count over its vertices
    sched = np.zeros((NT, NW), dtype=np.int32)
    for t in range(nfull):
        sched[t] = cnt[order[t * NCORES * P : (t + 1) * NCORES * P]].max(axis=0)
    sched[nfull] = cnt[order[nfull * NCORES * P :]].max(axis=0)

    offC = np.zeros((NT, NW), dtype=np.int32)
    offC[:, 1:] = np.cumsum(sched, axis=1)[:, :-1]
    cts = sched.sum(axis=1)

    karange = np.arange(K)[None, :]
    idxbs = []
    for c in range(NCORES):
        parts = []
        for t in range(NT):
            rows = min(P, VS - t * P)
            verts = dev2orig[c, t * P : t * P + rows]
            aw = win32[verts]  # [rows, K]
            al = loc[verts]
            o2 = np.argsort(aw, axis=1, kind="stable")
            sw = np.take_along_axis(aw, o2, 1)
            sl = np.take_along_axis(al, o2, 1)
            rcnt = cnt[verts]
            starts = np.zeros((rows, NW), dtype=np.int32)
            starts[:, 1:] = np.cumsum(rcnt, axis=1)[:, :-1]
            rank = karange - np.take_along_axis(starts, sw, 1)
            col = offC[t][sw] + rank
            ct = int(cts[t])
            buf = np.zeros((ct, P), dtype=np.int16)
            flat = col * P + np.arange(rows)[:, None]
            buf.reshape(-1)[flat.reshape(-1)] = sl.reshape(-1)
            n = ct * P
            wrp = buf.reshape(n // 16, 16).T
            parts.append(np.tile(wrp, (8, 1)))
        idxbs.append(np.ascontiguousarray(np.concatenate(parts, axis=1)))
    return tab, sched, idxbs, dev2orig


def kernel(feat: np.ndarray, nidx: np.ndarray, **run_kwargs):
    assert feat.shape == (V, F), feat.shape
    assert nidx.shape == (V, K), nidx.shape
    feat = np.ascontiguousarray(feat, dtype=np.float32)
    nidx = np.ascontiguousarray(nidx.astype(np.int64, copy=False))

    tab, sched, idxbs, dev2orig = _prepare(feat, nidx)
    idx_total = idxbs[0].shape[1]
    nc = _get_prog(sched, idx_total)
    in_maps = [{"tab": tab, "idxb": idxbs[c]} for c in range(NCORES)]
    res = bass_utils.run_bass_kernel_spmd(
        nc, in_maps, core_ids=list(range(NCORES)), **run_kwargs
    )
    out = np.empty((V, 2 * F), dtype=np.float32)
    for c in range(NCORES):
        dev = res.results[c]["out"]
        out[dev2orig[c]] = dev
    out[:, 0:F] = out[:, 0:F] * (1.0 / K) - BIAS
    out[:, F:] -= BIAS
    if run_kwargs:
        return out, res
    return out


# revision 8
# speedup vs baseline: 1.0659x; 1.0659x over previous
"""AccumulateNeighbours (meanmax) Trainium2 kernel.

out[v] = concat(mean_k feat[nidx[v,k]], max_k feat[nidx[v,k]])  -> [V, 2F]

Design (see probes in git history of this session):
- Vertices sharded over 8 NeuronCores; fp16 feature table replicated per core.
- Gather uses the vectorized SWDGE `dma_gather` (InstDMAGatherAnt): int16
  indices => the 150k-row table is split into 5 windows of 30000 rows, each
  prefixed by an all-zero row (the padding target, local idx 0). Rows are
  padded to 256B (128 fp16). Values are stored as fp16(x + 8) so zero pads
  are neutral: sum gets +0 (corrected by the constant 32*8), max is dominated
  (x+8 > 0).
- Position coding: for a 128-vertex tile and window w, the index list is
  [C_w, 128] (list position c*128+p = neighbour c of vertex-in-partition p),
  padded per vertex to the tile max C_w. To shrink the padding, vertices are
  globally re-ordered (clustered by window-count profile) and dealt into
  tiles of 1024 (128 to each core) sharing one schedule; the host undoes the
  permutation on output.
- dma_gather is limited to 1024 indices (idx-read shape field): each window
  gather is chunked into <=8-column pieces. Chunks rotate over the 4 SWDGE
  queues; descriptor generation pipelines 4-wide across Q7 core pairs
  (~8ns/idx per pair, ~2ns/idx effective).
- Reductions: pairwise trees over the gathered [128, CT, 128] fp16 tile on
  the Vector engine (contiguous views), final level writes fp32 into the
  [128, 192] out tile. Host: mean = sum/32 - 8, max -= 8.

The schedule depends on the index data, so the device program is built per
input (compile is host-side, cached on the schedule bytes).
"""

import numpy as np

import concourse.bacc as bacc
import concourse.mybir as mybir
import concourse.tile as tile
from concourse import bass_utils

V, K, F = 150000, 32, 96
NCORES = 8
VS = V // NCORES  # 18750
P = 128
NT = (VS + P - 1) // P  # 147 (last tile 62 rows/core, 496-vertex supertile)
NW = 5
WROWS = 30000
TROWS = NW * (WROWS + 1)
RE = 128  # 256B rows
BIAS = 8.0
CMAX = 8  # max columns per dma_gather (1024 idx limit)

G_BUFS = 4
IDX_BUFS = 4
OUT_BUFS = 4
NQUEUES = 4

_prog_cache: dict = {}


def _build(sched: np.ndarray, idx_total: int):
    nc = bacc.Bacc(
        "TRN2", target_bir_lowering=False, debug=False, num_swdge_queues=NQUEUES
    )
    tab_d = nc.dram_tensor("tab", [TROWS, RE], mybir.dt.float16, kind="ExternalInput")
    idx_d = nc.dram_tensor("idxb", [P, idx_total], mybir.dt.int16, kind="ExternalInput")
    out_d = nc.dram_tensor("out", [VS, 2 * F], mybir.dt.float32, kind="ExternalOutput")

    tab_ap = tab_d.ap()
    idx_ap = idx_d.ap()
    out_ap = out_d.ap()

    cts = sched.sum(axis=1)
    ctm = int(cts.max())
    sa_w = ctm // 2 + 1
    sb_w = ctm // 4 + 2

    qctr = 0
    with tile.TileContext(nc) as tc:
        with (
            tc.tile_pool(name="idx", bufs=IDX_BUFS) as idx_pool,
            tc.tile_pool(name="g", bufs=G_BUFS) as g_pool,
            tc.tile_pool(name="sa", bufs=2) as sa_pool,
            tc.tile_pool(name="sb", bufs=2) as sb_pool,
            tc.tile_pool(name="ma", bufs=2) as ma_pool,
            tc.tile_pool(name="mb", bufs=2) as mb_pool,
            tc.tile_pool(name="o", bufs=OUT_BUFS) as o_pool,
        ):
            ioff = 0
            for t in range(NT):
                rows = min(P, VS - t * P)
                ct = int(cts[t])
                iw = 8 * ct
                idx_tile = idx_pool.tile([P, iw], mybir.dt.int16)
                nc.sync.dma_start(idx_tile[:, :], idx_ap[:, ioff : ioff + iw])

                g_tile = g_pool.tile([P, ct, RE], mybir.dt.float16)
                off = 0
                for w in range(NW):
                    cw = int(sched[t, w])
                    done = 0
                    while done < cw:
                        cc = min(CMAX, cw - done)
                        n = cc * P
                        c0 = off + done
                        nc.gpsimd.dma_gather(
                            out_ap=g_tile[:, c0 : c0 + cc, :],
                            in_ap=tab_ap[w * (WROWS + 1) : (w + 1) * (WROWS + 1), :],
                            idxs_ap=idx_tile[:, c0 * 8 : (c0 + cc) * 8],
                            num_idxs=n,
                            num_idxs_reg=n,
                            elem_size=RE,
                            elem_step=RE,
                            queue_num=qctr % NQUEUES,
                            single_packet=False,
                        )
                        qctr += 1
                        done += cc
                    off += cw

                o_tile = o_pool.tile([P, 2 * F], mybir.dt.float32)

                # sum tree (exact pairing; odd level copies the straggler)
                sa = sa_pool.tile([P, sa_w, RE], mybir.dt.float16)
                sb = sb_pool.tile([P, sb_w, RE], mybir.dt.float16)
                cur, nxt = None, sa
                n = ct
                while n > 2:
                    src = g_tile if cur is None else cur
                    h = n // 2
                    nc.vector.tensor_add(
                        nxt[:, 0:h, :], src[:, 0:h, :], src[:, h : 2 * h, :]
                    )
                    if n % 2:
                        nc.scalar.copy(
                            nxt[:, h : h + 1, :], src[:, 2 * h : 2 * h + 1, :]
                        )
                    n = h + (n % 2)
                    cur, nxt = nxt, (sb if nxt is sa else sa)
                src = g_tile if cur is None else cur
                if n == 2:
                    nc.vector.tensor_add(
                        o_tile[:, 0:F], src[:, 0, 0:F], src[:, 1, 0:F]
                    )
                else:
                    nc.scalar.copy(o_tile[:, 0:F], src[:, 0, 0:F])

                # max tree (overlap pairing handles odd)
                ma = ma_pool.tile([P, sa_w, RE], mybir.dt.float16)
                mb = mb_pool.tile([P, sb_w, RE], mybir.dt.float16)
                cur, nxt = None, ma
                n = ct
                while n > 2:
                    src = g_tile if cur is None else cur
                    h = (n + 1) // 2
                    nc.vector.tensor_max(
                        nxt[:, 0:h, :], src[:, 0:h, :], src[:, n - h : n, :]
                    )
                    n = h
                    cur, nxt = nxt, (mb if nxt is ma else ma)
                src = g_tile if cur is None else cur
                if n == 2:
                    nc.vector.tensor_max(
                        o_tile[:, F : 2 * F], src[:, 0, 0:F], src[:, 1, 0:F]
                    )
                else:
                    nc.scalar.copy(o_tile[:, F : 2 * F], src[:, 0, 0:F])

                nc.sync.dma_start(out_ap[t * P : t * P + rows, :], o_tile[:rows, :])
                ioff += iw
    nc.compile()
    return nc


def _get_prog(sched: np.ndarray, idx_total: int):
    key = (sched.tobytes(), idx_total, G_BUFS, IDX_BUFS, NQUEUES, CMAX, "v4sp")
    if key not in _prog_cache:
        _prog_cache[key] = _build(sched, idx_total)
    return _prog_cache[key]


def _prepare(feat: np.ndarray, nidx: np.ndarray):
    tab = np.zeros((TROWS, RE), dtype=np.float16)
    f16 = (feat + BIAS).astype(np.float16)
    for w in range(NW):
        tab[w * (WROWS + 1) + 1 : (w + 1) * (WROWS + 1), :F] = f16[
            w * WROWS : (w + 1) * WROWS
        ]

    win32 = (nidx // WROWS).astype(np.int32)  # [V, K]
    loc = (nidx - win32.astype(np.int64) * WROWS + 1).astype(np.int16)

    cnt = np.stack([(win32 == w).sum(axis=1) for w in range(NW)], axis=1).astype(
        np.int32
    )  # [V, NW]

    # cluster vertices by window profile; deal into supertiles of 1024
    snd = np.sort(cnt, axis=1)[:, -2]
    key = cnt.argmax(axis=1) * 10000 + cnt.max(axis=1) * 100 + snd
    order = np.argsort(key, kind="stable")  # dev position -> original vertex

    # dev2orig[c, r] = original vertex of core c, device row r
    dev2orig = np.empty((NCORES, VS), dtype=np.int64)
    nfull = VS // P  # 146 full tiles
    tail = VS - nfull * P  # 62
    for t in range(nfull):
        blk = order[t * NCORES * P : (t + 1) * NCORES * P]
        for c in range(NCORES):
            dev2orig[c, t * P : (t + 1) * P] = blk[c * P : (c + 1) * P]
    blk = order[nfull * NCORES * P :]
    for c in range(NCORES):
        dev2orig[c, nfull * P :] = blk[c * tail : (c + 1) * tail]

    # schedule: per supertile, max count over its vertices
    sched = np.zeros((NT, NW), dtype=np.int32)
    for t in range(nfull):
        sched[t] = cnt[order[t * NCORES * P : (t + 1) * NCORES * P]].max(axis=0)
    sched[nfull] = cnt[order[nfull * NCORES * P :]].max(axis=0)

    offC = np.zeros((NT, NW), dtype=np.int32)
    offC[:, 1:] = np.cumsum(sched, axis=1)[:, :-1]
    cts = sched.sum(axis=1)

    karange = np.arange(K)[None, :]
    idxbs = []
    for c in range(NCORES):
        parts = []
        for t in range(NT):
            rows = min(P, VS - t * P)
            verts = dev2orig[c, t * P : t * P + rows]
            aw = win32[verts]  # [rows, K]
            al = loc[verts]
            o2 = np.argsort(aw, axis=1, kind="stable")
            sw = np.take_along_axis(aw, o2, 1)
            sl = np.take_along_axis(al, o2, 1)
            rcnt = cnt[verts]
            starts = np.zeros((rows, NW), dtype=np.int32)
            starts[:, 1:] = np.cumsum(rcnt, axis=1)[:, :-1]
            rank = karange - np.take_along_axis(starts, sw, 1)
            col = offC[t][sw] + rank
            ct = int(cts[t])
            buf = np.zeros((ct, P), dtype=np.int16)
            flat = col * P + np.arange(rows)[:, None]
            buf.reshape(-1)[flat.reshape(-1)] = sl.reshape(-1)
            n = ct * P
            wrp = buf.reshape(n // 16, 16).T
            parts.append(np.tile(wrp, (8, 1)))
        idxbs.append(np.ascontiguousarray(np.concatenate(parts, axis=1)))
    return tab, sched, idxbs, dev2orig


def kernel(feat: np.ndarray, nidx: np.ndarray, **run_kwargs):
    assert feat.shape == (V, F), feat.shape
    assert nidx.shape == (V, K), nidx.shape
    feat = np.ascontiguousarray(feat, dtype=np.float32)
    nidx = np.ascontiguousarray(nidx.astype(np.int64, copy=False))

    tab, sched, idxbs, dev2orig = _prepare(feat, nidx)
    idx_total = idxbs[0].shape[1]
    nc = _get_prog(sched, idx_total)
    in_maps = [{"tab": tab, "idxb": idxbs[c]} for c in range(NCORES)]
    res = bass_utils.run_bass_kernel_spmd(
        nc, in_maps, core_ids=list(range(NCORES)), **run_kwargs
    )
    out = np.empty((V, 2 * F), dtype=np.float32)
    for c in range(NCORES):
        dev = res.results[c]["out"]
        out[dev2orig[c]] = dev
    out[:, 0:F] = out[:, 0:F] * (1.0 / K) - BIAS
    out[:, F:] -= BIAS
    if run_kwargs:
        return out, res
    return out


# revision 9
# speedup vs baseline: 1.2388x; 1.1622x over previous
"""AccumulateNeighbours (meanmax) Trainium2 kernel.

out[v] = concat(mean_k feat[nidx[v,k]], max_k feat[nidx[v,k]])  -> [V, 2F]

Design (see probes in git history of this session):
- Vertices sharded over 8 NeuronCores; fp16 feature table replicated per core.
- Gather uses the vectorized SWDGE `dma_gather` (InstDMAGatherAnt): int16
  indices => the 150k-row table is split into 5 windows of 30000 rows, each
  prefixed by an all-zero row (the padding target, local idx 0). Rows are
  padded to 256B (128 fp16). Values are stored as fp16(x + 8) so zero pads
  are neutral: sum gets +0 (corrected by the constant 32*8), max is dominated
  (x+8 > 0).
- Position coding: for a 128-vertex tile and window w, the index list is
  [C_w, 128] (list position c*128+p = neighbour c of vertex-in-partition p),
  padded per vertex to the tile max C_w. To shrink the padding, vertices are
  globally re-ordered (clustered by window-count profile) and dealt into
  tiles of 1024 (128 to each core) sharing one schedule; the host undoes the
  permutation on output.
- dma_gather is limited to 1024 indices (idx-read shape field): each window
  gather is chunked into <=8-column pieces. Chunks rotate over the 4 SWDGE
  queues; descriptor generation pipelines 4-wide across Q7 core pairs
  (~8ns/idx per pair, ~2ns/idx effective).
- Reductions: pairwise trees over the gathered [128, CT, 128] fp16 tile on
  the Vector engine (contiguous views), final level writes fp32 into the
  [128, 192] out tile. Host: mean = sum/32 - 8, max -= 8.

The schedule depends on the index data, so the device program is built per
input (compile is host-side, cached on the schedule bytes).
"""

import numpy as np

import concourse.bacc as bacc
import concourse.mybir as mybir
import concourse.tile as tile
from concourse import bass_utils

V, K, F = 150000, 32, 96
NCORES = 8
VS = V // NCORES  # 18750
P = 128
NT = (VS + P - 1) // P  # 147 (last tile 62 rows/core, 496-vertex supertile)
NW = 5
WROWS = 30000
TROWS = NW * (WROWS + 1)
RE = 128  # 256B rows
BIAS = 8.0
CMAX = 8  # max columns per dma_gather (1024 idx limit)

G_BUFS = 6
IDX_BUFS = 4
OUT_BUFS = 4
NQUEUES = 4

_prog_cache: dict = {}


def _build(sched: np.ndarray, idx_total: int):
    nc = bacc.Bacc(
        "TRN2", target_bir_lowering=False, debug=False, num_swdge_queues=NQUEUES
    )
    tab_d = nc.dram_tensor("tab", [TROWS, RE], mybir.dt.float16, kind="ExternalInput")
    idx_d = nc.dram_tensor("idxb", [P, idx_total], mybir.dt.int16, kind="ExternalInput")
    out_d = nc.dram_tensor("out", [VS, 2 * F], mybir.dt.float32, kind="ExternalOutput")

    tab_ap = tab_d.ap()
    idx_ap = idx_d.ap()
    out_ap = out_d.ap()

    cts = sched.sum(axis=1)
    ctm = int(cts.max())
    sa_w = ctm // 2 + 1
    sb_w = ctm // 4 + 2

    qctr = 0
    with tile.TileContext(nc) as tc:
        with (
            tc.tile_pool(name="idx", bufs=IDX_BUFS) as idx_pool,
            tc.tile_pool(name="g", bufs=G_BUFS) as g_pool,
            tc.tile_pool(name="sa", bufs=2) as sa_pool,
            tc.tile_pool(name="sb", bufs=2) as sb_pool,
            tc.tile_pool(name="ma", bufs=2) as ma_pool,
            tc.tile_pool(name="mb", bufs=2) as mb_pool,
            tc.tile_pool(name="o", bufs=OUT_BUFS) as o_pool,
        ):
            ioff = 0
            for t in range(NT):
                rows = min(P, VS - t * P)
                ct = int(cts[t])
                iw = 8 * ct
                idx_tile = idx_pool.tile([P, iw], mybir.dt.int16)
                nc.sync.dma_start(idx_tile[:, :], idx_ap[:, ioff : ioff + iw])

                g_tile = g_pool.tile([P, ct, RE], mybir.dt.float16)
                off = 0
                for w in range(NW):
                    cw = int(sched[t, w])
                    nch = -(-cw // CMAX) if cw else 0
                    done = 0
                    for ci in range(nch):
                        cc = cw // nch + (1 if ci < cw % nch else 0)
                        n = cc * P
                        c0 = off + done
                        nc.gpsimd.dma_gather(
                            out_ap=g_tile[:, c0 : c0 + cc, :],
                            in_ap=tab_ap[w * (WROWS + 1) : (w + 1) * (WROWS + 1), :],
                            idxs_ap=idx_tile[:, c0 * 8 : (c0 + cc) * 8],
                            num_idxs=n,
                            num_idxs_reg=n,
                            elem_size=RE,
                            elem_step=RE,
                            queue_num=qctr % NQUEUES,
                            single_packet=False,
                        )
                        qctr += 1
                        done += cc
                    off += cw

                o_tile = o_pool.tile([P, 2 * F], mybir.dt.float32)

                # sum tree (exact pairing; odd level copies the straggler)
                sa = sa_pool.tile([P, sa_w, RE], mybir.dt.float16)
                sb = sb_pool.tile([P, sb_w, RE], mybir.dt.float16)
                cur, nxt = None, sa
                n = ct
                while n > 2:
                    src = g_tile if cur is None else cur
                    h = n // 2
                    nc.vector.tensor_add(
                        nxt[:, 0:h, :], src[:, 0:h, :], src[:, h : 2 * h, :]
                    )
                    if n % 2:
                        nc.scalar.copy(
                            nxt[:, h : h + 1, :], src[:, 2 * h : 2 * h + 1, :]
                        )
                    n = h + (n % 2)
                    cur, nxt = nxt, (sb if nxt is sa else sa)
                src = g_tile if cur is None else cur
                if n == 2:
                    nc.vector.tensor_add(
                        o_tile[:, 0:F], src[:, 0, 0:F], src[:, 1, 0:F]
                    )
                else:
                    nc.scalar.copy(o_tile[:, 0:F], src[:, 0, 0:F])

                # max tree (overlap pairing handles odd)
                ma = ma_pool.tile([P, sa_w, RE], mybir.dt.float16)
                mb = mb_pool.tile([P, sb_w, RE], mybir.dt.float16)
                cur, nxt = None, ma
                n = ct
                while n > 2:
                    src = g_tile if cur is None else cur
                    h = (n + 1) // 2
                    nc.vector.tensor_max(
                        nxt[:, 0:h, :], src[:, 0:h, :], src[:, n - h : n, :]
                    )
                    n = h
                    cur, nxt = nxt, (mb if nxt is ma else ma)
                src = g_tile if cur is None else cur
                if n == 2:
                    nc.vector.tensor_max(
                        o_tile[:, F : 2 * F], src[:, 0, 0:F], src[:, 1, 0:F]
                    )
                else:
                    nc.scalar.copy(o_tile[:, F : 2 * F], src[:, 0, 0:F])

                nc.sync.dma_start(out_ap[t * P : t * P + rows, :], o_tile[:rows, :])
                ioff += iw
    nc.compile()
    return nc


def _get_prog(sched: np.ndarray, idx_total: int):
    key = (sched.tobytes(), idx_total, G_BUFS, IDX_BUFS, NQUEUES, CMAX, "v6even")
    if key not in _prog_cache:
        _prog_cache[key] = _build(sched, idx_total)
    return _prog_cache[key]


def _prepare(feat: np.ndarray, nidx: np.ndarray):
    tab = np.zeros((TROWS, RE), dtype=np.float16)
    f16 = (feat + BIAS).astype(np.float16)
    for w in range(NW):
        tab[w * (WROWS + 1) + 1 : (w + 1) * (WROWS + 1), :F] = f16[
            w * WROWS : (w + 1) * WROWS
        ]

    win32 = (nidx // WROWS).astype(np.int32)  # [V, K]
    loc = (nidx - win32.astype(np.int64) * WROWS + 1).astype(np.int16)

    cnt = np.stack([(win32 == w).sum(axis=1) for w in range(NW)], axis=1).astype(
        np.int32
    )  # [V, NW]

    # cluster vertices by window profile; deal into supertiles of 1024
    snd = np.sort(cnt, axis=1)[:, -2]
    key = cnt.argmax(axis=1) * 10000 + cnt.max(axis=1) * 100 + snd
    order = np.argsort(key, kind="stable")  # dev position -> original vertex

    # dev2orig[c, r] = original vertex of core c, device row r
    dev2orig = np.empty((NCORES, VS), dtype=np.int64)
    nfull = VS // P  # 146 full tiles
    tail = VS - nfull * P  # 62
    for t in range(nfull):
        blk = order[t * NCORES * P : (t + 1) * NCORES * P]
        for c in range(NCORES):
            dev2orig[c, t * P : (t + 1) * P] = blk[c * P : (c + 1) * P]
    blk = order[nfull * NCORES * P :]
    for c in range(NCORES):
        dev2orig[c, nfull * P :] = blk[c * tail : (c + 1) * tail]

    # schedule: per supertile, max count over its vertices
    sched = np.zeros((NT, NW), dtype=np.int32)
    for t in range(nfull):
        sched[t] = cnt[order[t * NCORES * P : (t + 1) * NCORES * P]].max(axis=0)
    sched[nfull] = cnt[order[nfull * NCORES * P :]].max(axis=0)

    offC = np.zeros((NT, NW), dtype=np.int32)
    offC[:, 1:] = np.cumsum(sched, axis=1)[:, :-1]
    cts = sched.sum(axis=1)

    karange = np.arange(K)[None, :]
    idxbs = []
    for c in range(NCORES):
        parts = []
        for t in range(NT):
            rows = min(P, VS - t * P)
            verts = dev2orig[c, t * P : t * P + rows]
            aw = win32[verts]  # [rows, K]
            al = loc[verts]
            o2 = np.argsort(aw, axis=1, kind="stable")
            sw = np.take_along_axis(aw, o2, 1)
            sl = np.take_along_axis(al, o2, 1)
            rcnt = cnt[verts]
            starts = np.zeros((rows, NW), dtype=np.int32)
            starts[:, 1:] = np.cumsum(rcnt, axis=1)[:, :-1]
            rank = karange - np.take_along_axis(starts, sw, 1)
            col = offC[t][sw] + rank
            ct = int(cts[t])
            buf = np.zeros((ct, P), dtype=np.int16)
            flat = col * P + np.arange(rows)[:, None]
            buf.reshape(-1)[flat.reshape(-1)] = sl.reshape(-1)
            n = ct * P
            wrp = buf.reshape(n // 16, 16).T
            parts.append(np.tile(wrp, (8, 1)))
        idxbs.append(np.ascontiguousarray(np.concatenate(parts, axis=1)))
    return tab, sched, idxbs, dev2orig


def kernel(feat: np.ndarray, nidx: np.ndarray, **run_kwargs):
    assert feat.shape == (V, F), feat.shape
    assert nidx.shape == (V, K), nidx.shape
    feat = np.ascontiguousarray(feat, dtype=np.float32)
    nidx = np.ascontiguousarray(nidx.astype(np.int64, copy=False))

    tab, sched, idxbs, dev2orig = _prepare(feat, nidx)
    idx_total = idxbs[0].shape[1]
    nc = _get_prog(sched, idx_total)
    in_maps = [{"tab": tab, "idxb": idxbs[c]} for c in range(NCORES)]
    res = bass_utils.run_bass_kernel_spmd(
        nc, in_maps, core_ids=list(range(NCORES)), **run_kwargs
    )
    out = np.empty((V, 2 * F), dtype=np.float32)
    for c in range(NCORES):
        dev = res.results[c]["out"]
        out[dev2orig[c]] = dev
    out[:, 0:F] = out[:, 0:F] * (1.0 / K) - BIAS
    out[:, F:] -= BIAS
    if run_kwargs:
        return out, res
    return out
